# revision 1
# baseline (speedup 1.0000x reference)
"""Trainium2 Bass kernel for CausalAttentionSortNet (bucketed causal sort-net scores).

Math (per bh slice; n=8192, bucket=64, nb=128 buckets, d=64):
  sq[i]  = cumavg(q)[64*i] / 8    = c8[i] * (q[64i] + sum_{j<i} qb[j])
  sk[j]  = H[j] * sum_{j'<j} kb[j'] + sum_s G[j,s] k[64j+s]
  R[i,jj] = sq[i] . skp[jj] ; skp = [0, sk[0..126]] padded front
  masked softmax over jj<=i, then keep strictly jj<i.

v2 layout: per bh, DMA q/k bucket-contiguous as [128 j-partitions, 4096]
(16KB contiguous HBM per partition). The bucket sums fused with the causal
prefix combine run on PE: 8 accumulating 512-col matmuls per tensor with the
prefix weight matrix as stationary (psX[i, r] = sum_j pm[j,i] sum_c t[j,
512c+r]), followed by one short strided DVE reduce over the 8 residual
s-groups. The G-weighted sum for k keeps an elementwise multiply (GPSIMD,
reading the [128,64] G table with a d-broadcast AP) + chunked DVE reduces.
Scores are a small fp16 matmul onto a PSUM bank preloaded with the causal
mask; softmax skips max-subtraction (|logits| < 3).
"""

import numpy as np
from contextlib import ExitStack

import concourse.bass as bass
import concourse.tile as tile
from concourse import mybir
from concourse import bass_utils

# ---------------- problem constants (hardcoded per spec) ----------------
BH_TOTAL = 32
N_CORES = 8
BH = BH_TOTAL // N_CORES          # 4 bh slices per core
SEQ = 8192
D = 64
BUCKET = 64
NB = SEQ // BUCKET                # 128 buckets
NJ = NB + 1                       # 129 output cols
NEG = -1e30

_F32 = mybir.dt.float32
_F32R = mybir.dt.float32r
_F16 = mybir.dt.float16

KCH = 1024                        # k DMA / gpsimd chunk (cols)
MM = 512                          # matmul moving width (one PSUM bank)
QCH = (2048, 1536, 384, 128)      # q DMA chunks (cols)


def _host_constants():
    inv = 1.0 / np.arange(1, SEQ + 1, dtype=np.float64)          # 1/(t+1)
    invb = inv.reshape(NB, BUCKET)                               # [j, s]
    H = invb.sum(axis=1)                                         # [128]
    # suffix sums within bucket: G[j, s] = sum_{s'>=s} inv[j, s']
    G = np.cumsum(invb[:, ::-1], axis=1)[:, ::-1]                # [128, 64]

    i_idx = np.arange(NB)
    c8 = 1.0 / (8.0 * (BUCKET * i_idx + 1))                      # c_i/8
    j_col = i_idx[:, None]
    i_row = i_idx[None, :]
    pmq = np.where(j_col < i_row, c8[None, :], 0.0)              # [j, i]
    pmk = np.where(j_col < i_row, H[None, :], 0.0)               # [j, j2]

    ident = np.eye(128)

    jj_col = np.arange(NJ)[None, :]
    i_rows = np.arange(NB)[:, None]
    maskneg = np.where(jj_col <= i_rows, 0.0, NEG)               # [128, 129]
    maskstrict = (jj_col < i_rows).astype(np.float64)            # [128, 129]

    f = np.float32
    # cpackr is consumed as float32r (matmul stationaries: pmq, pmk and the
    # identity for the kw-sum); cpackf as plain f32. Separate tensors because
    # the BIR verifier checks producer/consumer fp32r-ness per memory
    # location.
    cpackr = np.concatenate([pmq, pmk, ident], axis=1)
    cpackf = np.concatenate([
        c8.reshape(128, 1), ident, maskneg, maskstrict,
    ], axis=1)
    return dict(gsmall=G.astype(f), cpackr=cpackr.astype(f),
                cpackf=cpackf.astype(f))


def _build_program():
    nc = bass.Bass("TRN2", target_bir_lowering=False, debug=False)

    q_t = nc.dram_tensor("q", [BH, SEQ, D], _F32, kind="ExternalInput")
    k_t = nc.dram_tensor("k", [BH, SEQ, D], _F32, kind="ExternalInput")
    g_t = nc.dram_tensor("gsmall", [128, 64], _F32, kind="ExternalInput")
    cpr_t = nc.dram_tensor("cpackr", [128, 384], _F32, kind="ExternalInput")
    cpf_t = nc.dram_tensor("cpackf", [128, 387], _F32, kind="ExternalInput")
    out_t = nc.dram_tensor("out", [BH, NB, NJ], _F32, kind="ExternalOutput")

    with tile.TileContext(nc) as tc, ExitStack() as ctx:
        _body(ctx, tc, q_t.ap(), k_t.ap(), out_t.ap(), g_t.ap(),
              cpr_t.ap(), cpf_t.ap())
    _split_matmul_waits(nc)
    return nc


_NO_SPLIT = ()


def _split_matmul_waits(nc):
    """This walrus build rejects compute instructions carrying more than one
    sync wait. Moving the waits onto single-wait NoOps placed immediately
    before the instruction in the same engine queue is semantically
    identical: the sequencer executes waits in queue order before
    dispatching."""
    n = 0
    for f in nc.m.functions:
        for b in f.blocks:
            insts = list(b.instructions)
            out = []
            changed = False
            for i in insts:
                si = getattr(i, "sync_info", None)
                if (si is not None and len(si.on_wait) > 1
                        and type(i).__name__ not in _NO_SPLIT
                        and i.is_executable()):
                    n += 1
                    changed = True
                    for wi, w in enumerate(si.on_wait):
                        nop = mybir.InstNoOp(
                            name=f"{i.name}-wsplit{wi}", ins=[], outs=[])
                        nop.engine = i.engine
                        nop.sync_info = mybir.SyncInfo(on_wait=[w], on_update=[])
                        out.append(nop)
                    i.sync_info = mybir.SyncInfo(
                        on_wait=[], on_update=list(si.on_update))
                out.append(i)
            if changed:
                b.instructions = out
    return n


def _body(ctx, tc, q, k, out, g_d, cpr_d, cpf_d):
    nc = tc.nc
    cpool = ctx.enter_context(tc.tile_pool(name="consts", bufs=1))
    dpool = ctx.enter_context(tc.tile_pool(name="data", bufs=2))
    spool = ctx.enter_context(tc.tile_pool(name="small", bufs=2))
    ppool = ctx.enter_context(tc.tile_pool(name="psum", bufs=2, space="PSUM"))

    # resident constants; the tile handles are created up front but the DMAs
    # are issued after the first k chunk (big transfer) so the small const
    # transfers pipeline behind it without a stream bubble
    gs = cpool.tile([128, 64], _F32, tag="gsmall")
    cpr = cpool.tile([128, 384], _F32R, tag="cpackr")
    cpf = cpool.tile([128, 387], _F32, tag="cpackf")

    def load_consts():
        nc.sync.dma_start(cpr[:], cpr_d.bitcast(_F32R))
        nc.sync.dma_start(cpf[:], cpf_d)
    pmq = cpr[:, 0:128]
    pmk = cpr[:, 128:256]
    identr = cpr[:, 256:384]
    cq8 = cpf[:, 0:1]
    ident = cpf[:, 1:129]
    maskneg = cpf[:, 129:258]
    maskstrict = cpf[:, 258:387]

    def vds(t, s):
        # [128, s*64] -> [128, d, s] view for the strided s-reduce
        return t.rearrange("j (s d) -> j d s", s=s, d=64)

    for bh in range(BH):
        # ---- k loads (4 chunks), gpsimd multiplies, PE prefix matmuls ----
        kt = dpool.tile([128, 4096], _F32, tag="kt", bufs=2)
        ksrc = k[bh].rearrange("(j r) d -> j (r d)", r=64)
        for c in range(4):
            sl = slice(KCH * c, KCH * (c + 1))
            if bh == 0 and c == 0:
                nc.sync.dma_start(gs[:], g_d)
            nc.sync.dma_start(kt[:, sl].bitcast(_F32R),
                              ksrc[:, sl].bitcast(_F32R))
            if bh == 0 and c == 0:
                load_consts()

        # kw = kt * G (d-broadcast): chunks 0,1 on GPSIMD, 2,3 on DVE so the
        # last chunk's multiply completes right after its DMA rather than
        # at the end of a serial 8.5us GPSIMD queue.
        kw = dpool.tile([128, 4096], _F32R, tag="kw", bufs=2)
        for c, eng in ((0, nc.gpsimd), (1, nc.gpsimd), (2, nc.vector),
                       (3, nc.vector)):
            sl = slice(KCH * c, KCH * (c + 1))
            gb = gs[:, 16 * c:16 * (c + 1)].unsqueeze(2).broadcast_to(
                [128, 16, 64])
            eng.tensor_mul(
                kw[:, sl].rearrange("j (s d) -> j s d", d=64),
                kt[:, sl].rearrange("j (s d) -> j s d", d=64),
                gb)

        # PE: k prefix (pmk) and kw-sum (identity) matmuls, emitted in data-
        # readiness order (kt chunk c arrives before mult c completes)
        psK = ppool.tile([128, MM], _F32, tag="psK", bufs=1)
        psW = ppool.tile([128, MM], _F32, tag="psW", bufs=1)

        def mmK(c):
            nc.tensor.matmul(psK[:], pmk, kt[:, MM * c:MM * (c + 1)].bitcast(_F32R),
                             start=(c == 0), stop=(c == 7))

        def mmW(c):
            nc.tensor.matmul(psW[:], identr, kw[:, MM * c:MM * (c + 1)],
                             start=(c == 0), stop=(c == 7))

        mmK(0); mmK(1); mmK(2); mmK(3)
        mmW(0); mmW(1)
        mmK(4); mmK(5)
        mmW(2); mmW(3)
        mmK(6); mmK(7)
        mmW(4); mmW(5); mmW(6); mmW(7)

        # ---- q loads ----
        qt = dpool.tile([128, 4096], _F32, tag="qt", bufs=2)
        qsrc = q[bh].rearrange("(j r) d -> j (r d)", r=64)
        o = 0
        for ln in QCH:
            nc.sync.dma_start(qt[:, o:o + ln].bitcast(_F32R),
                              qsrc[:, o:o + ln].bitcast(_F32R))
            o += ln

        # ---- early per-bh prep (DVE, no data deps) ----
        psS = ppool.tile([128, MM], _F32, tag="psS", bufs=1)
        skpT = spool.tile([64, NJ + 3], _F16, tag="skpT")
        nc.vector.memset(skpT[:, 0:1], 0.0)

        # ---- k-side combine (DVE) ----
        kpre = spool.tile([128, D], _F32, tag="kpre")
        nc.vector.reduce_sum(kpre[:], vds(psK[:], 8), axis=mybir.AxisListType.X)
        kg = spool.tile([128, D], _F32, tag="kg")
        nc.vector.reduce_sum(kg[:], vds(psW[:], 8), axis=mybir.AxisListType.X)
        sk = spool.tile([128, D], _F32, tag="sk")
        nc.vector.tensor_add(sk[:], kg[:], kpre[:])

        # ---- q prefix matmuls (PE) ----
        # execution order leaves psQ cols 128:512 final after the 384-col
        # matmul and cols 0:128 final after the last 128-col one, so the bq
        # reduce splits into an early part and a short tail part
        psQ = ppool.tile([128, MM], _F32, tag="psQ", bufs=1)
        qmm = [(0, 512), (512, 512), (1024, 512), (1536, 512),
               (2048, 512), (2560, 512), (3072, 512), (3584, 384),
               (3968, 128)]

        def mmQ(i):
            o, ln = qmm[i]
            nc.tensor.matmul(psQ[:, 0:ln], pmq, qt[:, o:o + ln].bitcast(_F32R),
                             start=(i == 0), stop=(i == 7))

        for i in range(4):
            mmQ(i)

        # ---- transpose k-side, stage fp16 skpT ----
        psT = ppool.tile([64, 256], _F32, tag="psT", bufs=1)
        nc.tensor.transpose(psT[0:64, 0:128], sk[:], ident)
        nc.scalar.copy(skpT[:, 1:129], psT[0:64, 0:128])

        for i in range(4, 8):
            mmQ(i)

        # ---- tail: q finish, scores, softmax, out ----
        # the first 8 matmuls close psQ's accumulation group, so the bulk bq
        # reduce runs before the last 128-col chunk even arrives; that chunk
        # gets its own small PSUM group and a short reduce in the tail
        bq_hi = spool.tile([128, D], _F32, tag="bq_hi")
        nc.vector.reduce_sum(bq_hi[:], vds(psQ[:], 8),
                             axis=mybir.AxisListType.X)
        sq_pre = spool.tile([128, D], _F32, tag="sq_pre")
        nc.vector.scalar_tensor_tensor(sq_pre[:], qt[:, 0:D], cq8, bq_hi[:],
                                       op0=mybir.AluOpType.mult,
                                       op1=mybir.AluOpType.add)
        psQ2 = ppool.tile([128, 128], _F32, tag="psQ2", bufs=1)
        o8, ln8 = qmm[8]
        nc.tensor.matmul(psQ2[:], pmq, qt[:, o8:o8 + ln8].bitcast(_F32R),
                         start=True, stop=True)
        bq_lo = spool.tile([128, D], _F32, tag="bq_lo")
        nc.vector.reduce_sum(bq_lo[:], vds(psQ2[:], 2),
                             axis=mybir.AxisListType.X)
        sq = spool.tile([128, D], _F32, tag="sq")
        nc.vector.tensor_add(sq[:], sq_pre[:], bq_lo[:])
        nc.tensor.transpose(psT[0:64, 128:256], sq[:], ident)
        sqT = spool.tile([64, 128], _F16, tag="sqT")
        nc.scalar.copy(sqT[:], psT[0:64, 128:256])

        nc.tensor.matmul(psS[:, 0:NJ], sqT[:], skpT[:, 0:NJ],
                         start=True, stop=True)
        Rm = spool.tile([128, NJ], _F32, tag="Rm")
        nc.vector.tensor_add(Rm[:], psS[:, 0:NJ], maskneg)

        e = spool.tile([128, NJ], _F32, tag="e")
        den = spool.tile([128, 1], _F32, tag="den")
        nc.scalar.activation(e[:], Rm[:],
                             mybir.ActivationFunctionType.Exp,
                             bias=0.0, scale=1.0, accum_out=den[:])
        rden = spool.tile([128, 1], _F32, tag="rden")
        nc.vector.reciprocal(rden[:], den[:])
        outb = spool.tile([128, NJ], _F32, tag="outb")
        nc.vector.scalar_tensor_tensor(outb[:], e[:], rden[:], maskstrict,
                                       op0=mybir.AluOpType.mult,
                                       op1=mybir.AluOpType.mult)
        # last slice's output goes via SP (shorter DGE path, nothing queued
        # behind it there); earlier ones via Act to keep SP's input stream
        # unblocked
        eng = nc.sync if bh == BH - 1 else nc.scalar
        eng.dma_start(out[bh], outb[:])


_CACHE = {}


def _get_program():
    if "nc" not in _CACHE:
        _CACHE["nc"] = _build_program()
        _CACHE["consts"] = _host_constants()
    return _CACHE["nc"], _CACHE["consts"]


def _get_runner():
    """Build the sharded PJRT callable once and cache it (mirrors
    bass2jax.run_bass_via_pjrt but reuses the jitted function across
    calls)."""
    if "runner" in _CACHE:
        return _CACHE["runner"]
    import jax
    from jax.sharding import Mesh, PartitionSpec
    from jax.experimental.shard_map import shard_map
    from concourse import bass2jax

    nc, consts = _get_program()
    bass2jax.install_neuronx_cc_hook()

    part_name = nc.partition_id_tensor.name if nc.partition_id_tensor else None
    in_names, out_names, out_avals, zero_outs = [], [], [], []
    for alloc in nc.m.functions[0].allocations:
        if not isinstance(alloc, mybir.MemoryLocationSet):
            continue
        name = alloc.memorylocations[0].name
        if alloc.kind == "ExternalInput":
            if name != part_name:
                in_names.append(name)
        elif alloc.kind == "ExternalOutput":
            out_names.append(name)
            shape = tuple(alloc.tensor_shape)
            dtype = mybir.dt.np(alloc.dtype)
            out_avals.append(jax.core.ShapedArray(shape, dtype))
            zero_outs.append(np.zeros(shape, dtype))
    n_params = len(in_names)
    all_names = in_names + out_names
    if part_name is not None:
        all_names = all_names + [part_name]
    donate = tuple(range(n_params, n_params + len(out_names)))

    def _body(*args):
        operands = list(args)
        if part_name is not None:
            operands.append(bass2jax.partition_id_tensor())
        outs = bass2jax._bass_exec_p.bind(
            *operands,
            out_avals=tuple(out_avals),
            in_names=tuple(all_names),
            out_names=tuple(out_names),
            lowering_input_output_aliases=(),
            sim_require_finite=True,
            sim_require_nnan=True,
            nc=nc,
        )
        return tuple(outs)

    devices = jax.devices()[:N_CORES]
    mesh = Mesh(np.asarray(devices), ("core",))
    specs = (PartitionSpec("core"),) * (n_params + len(out_names))
    sharded = jax.jit(
        shard_map(_body, mesh=mesh, in_specs=specs,
                  out_specs=(PartitionSpec("core"),) * len(out_names),
                  check_rep=False),
        donate_argnums=donate, keep_unused=True,
    )
    runner = dict(fn=sharded, in_names=in_names, out_names=out_names,
                  zero_outs=zero_outs, consts=consts, nc=nc)
    _CACHE["runner"] = runner
    return runner


def _concat_inputs(q, k, runner):
    """Per-core input dict -> globally concatenated arrays (axis 0)."""
    consts = runner["consts"]
    arrs = []
    for name in runner["in_names"]:
        if name == "q":
            arrs.append(q)
        elif name == "k":
            arrs.append(k)
        else:
            c = consts[name]
            arrs.append(np.concatenate([c] * N_CORES, axis=0))
    return arrs


def kernel(q, k):
    q = np.ascontiguousarray(np.asarray(q, dtype=np.float32))
    k = np.ascontiguousarray(np.asarray(k, dtype=np.float32))
    assert q.shape == (BH_TOTAL, SEQ, D) and k.shape == (BH_TOTAL, SEQ, D)

    runner = _get_runner()
    # bh-shard across 8 cores: core c gets bh slice [4c, 4c+4). The global
    # concat layout [32, ...] already matches (shard_map splits axis 0).
    concat_in = _concat_inputs(q, k, runner)
    concat_zeros = [np.zeros((N_CORES * z.shape[0], *z.shape[1:]), z.dtype)
                    for z in runner["zero_outs"]]
    out_arrs = runner["fn"](*concat_in, *concat_zeros)
    out = np.asarray(out_arrs[0])          # [8*4, 128, 129]
    return np.ascontiguousarray(out.reshape(BH_TOTAL, NB, NJ))



# revision 39
# speedup vs baseline: 1.0577x; 1.0577x over previous
"""Trainium2 Bass kernel for CausalAttentionSortNet (bucketed causal sort-net scores).

Math (per bh slice; n=8192, bucket=64, nb=128 buckets, d=64):
  sq[i]  = cumavg(q)[64*i] / 8    = c8[i] * (q[64i] + sum_{j<i} qb[j])
  sk[j]  = H[j] * sum_{j'<j} kb[j'] + sum_s G[j,s] k[64j+s]
  R[i,jj] = sq[i] . skp[jj] ; skp = [0, sk[0..126]] padded front
  masked softmax over jj<=i, then keep strictly jj<i.

v3: all constants (G, H, c8, prefix matrices, masks, identities) are
generated ON DEVICE at start (gpsimd iota/affine_select + small PE
matmuls) - no const DMA traffic, so HBM transfers are inputs+outputs
only. The causal -inf mask is preloaded into the score PSUM bank and the
score matmul accumulates onto it (start=False), removing the mask add
from the critical tail. The q side computes raw bucket sums qb via 15
accumulating 256-col matmuls (identity stationary) plus a short DVE
strided reduce for the final 256-col chunk, then one matmul
qb^T(stat) x pmq(moving) produces the c8-weighted causal prefix already
TRANSPOSED ([d, i]) for the fp16 score matmul - the post-last-chunk
chain is reduce/add -> matmul -> add(f16) -> matmul -> exp -> mul -> DMA.
"""

import numpy as np
from contextlib import ExitStack

import concourse.bass as bass
import concourse.tile as tile
from concourse import mybir
from concourse import bass_utils

# ---------------- problem constants (hardcoded per spec) ----------------
BH_TOTAL = 32
N_CORES = 8
BH = BH_TOTAL // N_CORES          # 4 bh slices per core
SEQ = 8192
D = 64
BUCKET = 64
NB = SEQ // BUCKET                # 128 buckets
NJ = NB + 1                       # 129 output cols
NEG = -1e30

_F32 = mybir.dt.float32
_F32R = mybir.dt.float32r
_F16 = mybir.dt.float16
_I32 = mybir.dt.int32

KCH = 1024                        # k DMA / gpsimd chunk (cols)
MM = 512                          # k matmul moving width (one PSUM bank)
QCH = (2048, 1024, 512, 256, 256)  # q DMA chunks (cols)
QF = 16                           # 256-col fold matmuls into psQf


def _build_program(split_waits=True):
    nc = bass.Bass("TRN2", target_bir_lowering=False, debug=False)

    q_t = nc.dram_tensor("q", [BH, SEQ, D], _F32, kind="ExternalInput")
    k_t = nc.dram_tensor("k", [BH, SEQ, D], _F32, kind="ExternalInput")
    out_t = nc.dram_tensor("out", [BH, NB, NJ], _F32, kind="ExternalOutput")

    with tile.TileContext(nc) as tc, ExitStack() as ctx:
        _body(ctx, tc, q_t.ap(), k_t.ap(), out_t.ap())
    _fix_prep_lane_sems(nc)
    if split_waits:
        _split_matmul_waits(nc)
    return nc


def _fix_prep_lane_sems(nc):
    """Tile's wait pass emits consumer/epilogue waits against the round-robin
    DMASW lane semaphores, but the completion increment SDMA actually fires
    is the prep's own `sem=` (on_update[0]), which nothing rewires - leaving
    the DMASW waits unsatisfiable. Each lane here carries exactly one prep,
    so point every DMASW-lane wait at that prep's completion sem instead."""
    lane_sem = {}
    k = 0
    for f in nc.m.functions:
        for b in f.blocks:
            for i in b.instructions:
                if type(i).__name__ in ("InstKVWritebackAnt",
                                        "InstPagedWritebackAnt",
                                        "InstDMAGatherAnt",
                                        "InstDMAScatterAddAnt") \
                        and getattr(i, "gen_mode", 0) == 1:
                    u0 = i.sync_info.on_update[0]
                    lane = f"DMASW{k % 8}"
                    assert lane not in lane_sem, "one prep per lane assumed"
                    lane_sem[lane] = (u0.id, u0.ant_name)
                    k += 1
    n = 0
    for f in nc.m.functions:
        for b in f.blocks:
            for i in b.instructions:
                si = getattr(i, "sync_info", None)
                if not si or not si.on_wait:
                    continue
                waits = list(si.on_wait)
                changed = False
                for wi, w in enumerate(waits):
                    name = (getattr(w, "ant_name", "") or "").split("_")[0]
                    if name in lane_sem:
                        sid, sname = lane_sem[name]
                        waits[wi] = mybir.SyncWait(
                            sync_type=w.sync_type, id=sid, ant_name=sname,
                            wait_mode=w.wait_mode, wait_value=w.wait_value,
                            wait_reg=None)
                        changed = True
                        n += 1
                if changed:
                    i.sync_info = mybir.SyncInfo(on_wait=waits,
                                                 on_update=list(si.on_update))
    return n


_NO_SPLIT = ()


def _split_matmul_waits(nc):
    """This walrus build rejects compute instructions carrying more than one
    sync wait. Moving the waits onto single-wait NoOps placed immediately
    before the instruction in the same engine queue is semantically
    identical: the sequencer executes waits in queue order before
    dispatching."""
    n = 0
    for f in nc.m.functions:
        for b in f.blocks:
            insts = list(b.instructions)
            out = []
            changed = False
            for i in insts:
                si = getattr(i, "sync_info", None)
                if (si is not None and len(si.on_wait) > 1
                        and type(i).__name__ not in _NO_SPLIT
                        and i.is_executable()):
                    n += 1
                    changed = True
                    for wi, w in enumerate(si.on_wait):
                        nop = mybir.InstNoOp(
                            name=f"{i.name}-wsplit{wi}", ins=[], outs=[])
                        nop.engine = i.engine
                        nop.sync_info = mybir.SyncInfo(on_wait=[w], on_update=[])
                        out.append(nop)
                    i.sync_info = mybir.SyncInfo(
                        on_wait=[], on_update=list(si.on_update))
                out.append(i)
            if changed:
                b.instructions = out
    return n


def _gen_consts(nc, cpool, ppool):
    """Generate all constants on device.

    invT[s, j] = 1/(64j + s + 1)  ->  H row + suffix-sum GT via PE matmuls
    c8row[i] = 1/(512 i + 8); pmq/pmk = causal-masked outer products.
    """
    AL = mybir.AluOpType

    # integer iotas (gpsimd is the only engine with iota/affine_select)
    it_inv = cpool.tile([64, 128], _I32, tag="it_inv")
    nc.gpsimd.iota(it_inv[:], pattern=[[64, 128]], base=1, channel_multiplier=1)
    it_c8 = cpool.tile([1, 128], _I32, tag="it_c8")
    nc.gpsimd.iota(it_c8[:], pattern=[[512, 128]], base=8, channel_multiplier=0)

    # float conversions + reciprocals (DVE)
    invTf = cpool.tile([64, 128], _F32, tag="invTf")
    nc.vector.tensor_copy(invTf[:], it_inv[:])
    invTs = cpool.tile([64, 128], _F32, tag="invTs")
    nc.vector.reciprocal(invTs[:], invTf[:])
    invT = cpool.tile([64, 128], _F32R, tag="invT")
    nc.vector.tensor_copy(invT[:], invTs[:])
    c8f = cpool.tile([1, 128], _F32, tag="c8f")
    nc.vector.tensor_copy(c8f[:], it_c8[:])
    c8rowf = cpool.tile([1, 128], _F32, tag="c8rowf")
    nc.vector.reciprocal(c8rowf[:], c8f[:])
    c8row = cpool.tile([1, 128], _F32R, tag="c8row")
    nc.vector.tensor_copy(c8row[:], c8rowf[:])

    # ones / triangular / identity
    onesf = cpool.tile([64, 1], _F32, tag="onesf")
    nc.vector.memset(onesf[:], 1.0)
    ones64 = cpool.tile([64, 1], _F32R, tag="ones64")
    nc.vector.tensor_copy(ones64[:], onesf[:])
    onesf1 = cpool.tile([1, 128], _F32, tag="onesf1")
    nc.vector.memset(onesf1[:], 1.0)
    ones1 = cpool.tile([1, 128], _F32R, tag="ones1")
    nc.vector.tensor_copy(ones1[:], onesf1[:])
    U64f = cpool.tile([64, 64], _F32, tag="U64f")        # U[s',s]=1 iff s'>=s
    nc.gpsimd.memset(U64f[:], 1.0)
    nc.gpsimd.affine_select(out=U64f[:], in_=U64f[:], compare_op=AL.is_ge,
                            fill=0.0, base=0, pattern=[[-1, 64]],
                            channel_multiplier=1)
    U64 = cpool.tile([64, 64], _F32R, tag="U64")
    nc.vector.tensor_copy(U64[:], U64f[:])
    ident = cpool.tile([128, 128], _F32, tag="ident")    # f32, for transposes
    nc.gpsimd.memset(ident[:], 0.0)
    nc.gpsimd.affine_select(out=ident[:], in_=ident[:], compare_op=AL.not_equal,
                            fill=1.0, base=0, pattern=[[-1, 128]],
                            channel_multiplier=1)
    identr = cpool.tile([128, 128], _F32R, tag="identr")  # f32r, stationary
    nc.vector.tensor_copy(identr[:], ident[:])

    masknegf = cpool.tile([128, NJ + 3], _F32, tag="masknegf")
    nc.gpsimd.memset(masknegf[:], 0.0)
    nc.gpsimd.affine_select(out=masknegf[:], in_=masknegf[:],
                            compare_op=AL.is_ge, fill=NEG, base=0,
                            pattern=[[-1, NJ + 3]], channel_multiplier=1)
    maskneg = cpool.tile([128, NJ + 3], _F32R, tag="maskneg")
    nc.vector.tensor_copy(maskneg[:], masknegf[:])
    maskstrict = cpool.tile([128, NJ], _F32, tag="maskstrict")
    nc.gpsimd.memset(maskstrict[:], 1.0)
    nc.gpsimd.affine_select(out=maskstrict[:], in_=maskstrict[:],
                            compare_op=AL.is_ge, fill=0.0, base=-1,
                            pattern=[[-1, NJ]], channel_multiplier=1)

    # PE-derived rows: H = col-sums of invT; GT = suffix sums of invT.
    # Scratch PSUM borrows the loop's psK/psW tags (it is done long before
    # the first k matmuls) so psK/psW can double-buffer within 8 banks.
    psC0 = ppool.tile([128, MM], _F32, tag="psK", bufs=2)
    psC1 = ppool.tile([128, MM], _F32, tag="psW", bufs=2)
    nc.tensor.matmul(psC0[0:1, 0:128], ones64[:], invT[:],
                     start=True, stop=True)
    Hrowf = cpool.tile([1, 128], _F32, tag="Hrowf")
    nc.vector.tensor_copy(Hrowf[:], psC0[0:1, 0:128])
    Hrow = cpool.tile([1, 128], _F32R, tag="Hrow")
    nc.vector.tensor_copy(Hrow[:], Hrowf[:])
    nc.tensor.matmul(psC1[0:64, 0:128], U64[:], invT[:],
                     start=True, stop=True)
    gtS = cpool.tile([64, 128], _F32, tag="gtS")
    nc.scalar.copy(gtS[:], psC1[0:64, 0:128])

    # pmq[j,i] = (j<i) c8[i]; pmk[j,i] = (j<i) H[i]   (outer product + mask)
    nc.tensor.matmul(psC0[:, 0:128], ones1[:], c8row[:],
                     start=True, stop=True)
    pmqf = cpool.tile([128, 128], _F32, tag="pmqf")
    nc.scalar.copy(pmqf[:], psC0[:, 0:128])
    nc.gpsimd.affine_select(out=pmqf[:], in_=pmqf[:], compare_op=AL.is_ge,
                            fill=0.0, base=-1, pattern=[[1, 128]],
                            channel_multiplier=-1)
    pmq = cpool.tile([128, 128], _F32R, tag="pmq")       # moving operand
    nc.vector.tensor_copy(pmq[:], pmqf[:])
    nc.tensor.matmul(psC1[:, 0:128], ones1[:], Hrow[:],
                     start=True, stop=True)
    pmkf = cpool.tile([128, 128], _F32, tag="pmkf")
    nc.scalar.copy(pmkf[:], psC1[:, 0:128])
    nc.gpsimd.affine_select(out=pmkf[:], in_=pmkf[:], compare_op=AL.is_ge,
                            fill=0.0, base=-1, pattern=[[1, 128]],
                            channel_multiplier=-1)
    pmk = cpool.tile([128, 128], _F32R, tag="pmk")       # stationary
    nc.vector.tensor_copy(pmk[:], pmkf[:])

    # G[j, s] = transpose(GT)
    nc.tensor.transpose(psC0[:, 0:64], gtS[:], ident[0:64, 0:64])
    gs = cpool.tile([128, 64], _F32, tag="gs")
    nc.vector.tensor_copy(gs[:], psC0[:, 0:64])

    # c8 broadcast along 64 partitions for the q0T scale (outer product)
    nc.tensor.matmul(psC1[0:64, 0:128], ones1[:, 0:64], c8row[:],
                     start=True, stop=True)
    c8bT = cpool.tile([64, 128], _F32, tag="c8bT")
    nc.vector.tensor_copy(c8bT[:], psC1[0:64, 0:128])

    return dict(gs=gs, ident=ident, identr=identr, pmq=pmq, pmk=pmk,
                c8bT=c8bT, maskneg=maskneg, maskstrict=maskstrict)


def _body(ctx, tc, q, k, out):
    nc = tc.nc
    cpool = ctx.enter_context(tc.tile_pool(name="consts", bufs=1))
    dpool = ctx.enter_context(tc.tile_pool(name="data", bufs=2))
    spool = ctx.enter_context(tc.tile_pool(name="small", bufs=2))
    ppool = ctx.enter_context(tc.tile_pool(name="psum", bufs=2, space="PSUM"))

    C = _gen_consts(nc, cpool, ppool)
    gs, ident, identr = C["gs"], C["ident"], C["identr"]
    pmq, pmk, c8bT = C["pmq"], C["pmk"], C["c8bT"]
    maskneg, maskstrict = C["maskneg"], C["maskstrict"]

    def vds(t, s):
        # [128, s*64] -> [128, d, s] view for the strided s-reduce
        return t.rearrange("j (s d) -> j d s", s=s, d=64)

    outbs = []
    for bh in range(BH):
        # ---- k loads (4 chunks), G-multiplies, PE prefix/fold matmuls ----
        kt = dpool.tile([128, 4096], _F32, tag="kt", bufs=2)
        ksrc = k[bh].rearrange("(j r) d -> j (r d)", r=64)
        for c in range(4):
            sl = slice(KCH * c, KCH * (c + 1))
            nc.sync.dma_start(kt[:, sl].bitcast(_F32R),
                              ksrc[:, sl].bitcast(_F32R))

        # kw = kt * G (d-broadcast): all chunks on DVE (faster per element
        # than GPSIMD), keeping the Pool queue free for the SWDGE output
        # preps/triggers - a Pool kw multiply queued behind a prep that
        # waits on the previous slice's softmax couples the k pipeline to
        # the previous tail through Pool's in-order engine ticks.
        kw = dpool.tile([128, 4096], _F32R, tag="kw", bufs=2)
        for c, eng in ((0, nc.vector), (1, nc.vector), (2, nc.vector),
                       (3, nc.vector)):
            sl = slice(KCH * c, KCH * (c + 1))
            gb = gs[:, 16 * c:16 * (c + 1)].unsqueeze(2).broadcast_to(
                [128, 16, 64])
            eng.tensor_mul(
                kw[:, sl].rearrange("j (s d) -> j s d", d=64),
                kt[:, sl].rearrange("j (s d) -> j s d", d=64),
                gb)

        # PE: k prefix (pmk) and kw-sum (identity) matmuls, emitted in data-
        # readiness order (kt chunk c arrives before mult c completes)
        psK = ppool.tile([128, MM], _F32, tag="psK", bufs=2)
        psW = ppool.tile([128, MM], _F32, tag="psW", bufs=2)

        def mmK(c):
            nc.tensor.matmul(psK[:], pmk[:], kt[:, MM * c:MM * (c + 1)].bitcast(_F32R),
                             start=(c == 0), stop=(c == 7))

        def mmW(c):
            nc.tensor.matmul(psW[:], identr[:], kw[:, MM * c:MM * (c + 1)],
                             start=(c == 0), stop=(c == 7))

        mmK(0); mmK(1); mmK(2); mmK(3)
        mmW(0); mmW(1)
        mmK(4); mmK(5)
        mmW(2); mmW(3)
        mmK(6); mmK(7)
        mmW(4); mmW(5); mmW(6); mmW(7)

        # ---- early per-bh prep (no data deps) ----
        psS = ppool.tile([128, MM], _F32, tag="psS", bufs=1)
        # causal -inf mask preloaded via a PE copy-matmul (identity
        # stationary x maskneg moving) that OPENS the psS accumulation
        # group; the tail's score matmul then accumulates onto it with
        # start=False. A DVE write into PSUM is not reliably visible to
        # the PE accumulation path on hardware.
        nc.tensor.matmul(psS[:, 0:NJ + 3], identr[:], maskneg[:],
                         start=True, stop=False)
        skpT = spool.tile([64, NJ + 3], _F16, tag="skpT")
        nc.vector.memset(skpT[:], 0.0)
        outb = spool.tile([128, NJ], _F32, tag="outb", bufs=4)
        outbs.append(outb)


        # ---- k-side combine (DVE) ----
        kpre = spool.tile([128, D], _F32, tag="kpre")
        nc.vector.reduce_sum(kpre[:], vds(psK[:], 8), axis=mybir.AxisListType.X)
        kg = spool.tile([128, D], _F32, tag="kg")
        nc.vector.reduce_sum(kg[:], vds(psW[:], 8), axis=mybir.AxisListType.X)
        sk = spool.tile([128, D], _F32, tag="sk")
        nc.vector.tensor_add(sk[:], kg[:], kpre[:])

        psT = ppool.tile([64, 256], _F32, tag="psT", bufs=1)
        nc.tensor.transpose(psT[0:64, 0:128], sk[:], ident[:])
        nc.scalar.copy(skpT[:, 1:NJ], psT[0:64, 0:128])

        # ---- q loads ----
        qt = dpool.tile([128, 4096], _F32, tag="qt", bufs=2)
        qsrc = q[bh].rearrange("(j r) d -> j (r d)", r=64)
        o = 0
        for ln in QCH:
            nc.sync.dma_start(qt[:, o:o + ln].bitcast(_F32R),
                              qsrc[:, o:o + ln].bitcast(_F32R))
            o += ln

        # ---- q bucket sums: psQf accumulates cols 0:3840 (15 x 256) ----
        psQf = ppool.tile([128, 256], _F32, tag="psQf", bufs=1)
        for m in range(8):
            nc.tensor.matmul(psQf[:], identr[:],
                             qt[:, 256 * m:256 * (m + 1)].bitcast(_F32R),
                             start=(m == 0), stop=False)
        # q0T while chunk 0 is resident; c8-scaled on DVE (off tail)
        nc.tensor.transpose(psT[0:64, 128:256], qt[:, 0:D], ident[0:128, :])
        c8q0T = spool.tile([64, 128], _F32, tag="c8q0T")
        nc.vector.tensor_mul(c8q0T[:], psT[0:64, 128:256], c8bT[:])
        for m in range(8, QF):
            nc.tensor.matmul(psQf[:], identr[:],
                             qt[:, 256 * m:256 * (m + 1)].bitcast(_F32R),
                             start=False, stop=(m == QF - 1))

        # ---- tail: close fold, prefix matmul, scores, softmax ----
        qb = spool.tile([128, D], _F32R, tag="qb")
        with nc.allow_low_precision(reason="f32r is bit-identical to f32"):
            nc.vector.reduce_sum(qb[:], vds(psQf[:], 4),
                                 axis=mybir.AxisListType.X)
        psBQT = ppool.tile([64, 128], _F32, tag="psBQT", bufs=1)
        nc.tensor.matmul(psBQT[:], qb[:], pmq[:],
                         start=True, stop=True)
        sqT16 = spool.tile([64, 128], _F16, tag="sqT16")
        nc.vector.tensor_add(sqT16[:], psBQT[:], c8q0T[:])
        nc.tensor.matmul(psS[:, 0:NJ + 3], sqT16[:], skpT[:],
                         start=False, stop=True)

        e = spool.tile([128, NJ], _F32, tag="e")
        den = spool.tile([128, 1], _F32, tag="den")
        nc.scalar.activation(e[:], psS[:, 0:NJ],
                             mybir.ActivationFunctionType.Exp,
                             bias=0.0, scale=1.0, accum_out=den[:])
        rden = spool.tile([128, 1], _F32, tag="rden")
        nc.vector.reciprocal(rden[:], den[:])
        nc.vector.scalar_tensor_tensor(outb[:], e[:], rden[:], maskstrict[:],
                                       op0=mybir.AluOpType.mult,
                                       op1=mybir.AluOpType.mult)

    # All output DMAs issue from SP AFTER the whole input stream: their
    # transfers land in the tail's natural DMA idle. Issuing them
    # mid-stream (e.g. from Act) parks them for microseconds behind the
    # saturated DMA engines, and the 8 round-robin HWDGE completion lanes
    # then make an input DMA eight slots later wait on the parked output.
    for bh, outb in enumerate(outbs):
        nc.sync.dma_start(out[bh], outb[:])


_CACHE = {}


def _get_program():
    if "nc" not in _CACHE:
        _CACHE["nc"] = _build_program()
        _CACHE["consts"] = {}
    return _CACHE["nc"], _CACHE["consts"]


def _get_runner():
    """Build the sharded PJRT callable once and cache it (mirrors
    bass2jax.run_bass_via_pjrt but reuses the jitted function across
    calls)."""
    if "runner" in _CACHE:
        return _CACHE["runner"]
    import jax
    from jax.sharding import Mesh, PartitionSpec
    from jax.experimental.shard_map import shard_map
    from concourse import bass2jax

    nc, consts = _get_program()
    bass2jax.install_neuronx_cc_hook()

    part_name = nc.partition_id_tensor.name if nc.partition_id_tensor else None
    in_names, out_names, out_avals, zero_outs = [], [], [], []
    for alloc in nc.m.functions[0].allocations:
        if not isinstance(alloc, mybir.MemoryLocationSet):
            continue
        name = alloc.memorylocations[0].name
        if alloc.kind == "ExternalInput":
            if name != part_name:
                in_names.append(name)
        elif alloc.kind == "ExternalOutput":
            out_names.append(name)
            shape = tuple(alloc.tensor_shape)
            dtype = mybir.dt.np(alloc.dtype)
            out_avals.append(jax.core.ShapedArray(shape, dtype))
            zero_outs.append(np.zeros(shape, dtype))
    n_params = len(in_names)
    all_names = in_names + out_names
    if part_name is not None:
        all_names = all_names + [part_name]
    donate = tuple(range(n_params, n_params + len(out_names)))

    def _body(*args):
        operands = list(args)
        if part_name is not None:
            operands.append(bass2jax.partition_id_tensor())
        outs = bass2jax._bass_exec_p.bind(
            *operands,
            out_avals=tuple(out_avals),
            in_names=tuple(all_names),
            out_names=tuple(out_names),
            lowering_input_output_aliases=(),
            sim_require_finite=True,
            sim_require_nnan=True,
            nc=nc,
        )
        return tuple(outs)

    devices = jax.devices()[:N_CORES]
    mesh = Mesh(np.asarray(devices), ("core",))
    specs = (PartitionSpec("core"),) * (n_params + len(out_names))
    sharded = jax.jit(
        shard_map(_body, mesh=mesh, in_specs=specs,
                  out_specs=(PartitionSpec("core"),) * len(out_names),
                  check_rep=False),
        donate_argnums=donate, keep_unused=True,
    )
    runner = dict(fn=sharded, in_names=in_names, out_names=out_names,
                  zero_outs=zero_outs, consts=consts, nc=nc)
    _CACHE["runner"] = runner
    return runner


def _concat_inputs(q, k, runner):
    """Per-core input dict -> globally concatenated arrays (axis 0)."""
    arrs = []
    for name in runner["in_names"]:
        if name == "q":
            arrs.append(q)
        elif name == "k":
            arrs.append(k)
        else:
            raise KeyError(name)
    return arrs


def kernel(q, k):
    q = np.ascontiguousarray(np.asarray(q, dtype=np.float32))
    k = np.ascontiguousarray(np.asarray(k, dtype=np.float32))
    assert q.shape == (BH_TOTAL, SEQ, D) and k.shape == (BH_TOTAL, SEQ, D)

    runner = _get_runner()
    # bh-shard across 8 cores: core c gets bh slice [4c, 4c+4). The global
    # concat layout [32, ...] already matches (shard_map splits axis 0).
    concat_in = _concat_inputs(q, k, runner)
    concat_zeros = [np.zeros((N_CORES * z.shape[0], *z.shape[1:]), z.dtype)
                    for z in runner["zero_outs"]]
    out_arrs = runner["fn"](*concat_in, *concat_zeros)
    out = np.asarray(out_arrs[0])          # [8*4, 128, 129]
    return np.ascontiguousarray(out.reshape(BH_TOTAL, NB, NJ))


# revision 40
# speedup vs baseline: 1.0584x; 1.0007x over previous
"""Trainium2 Bass kernel for CausalAttentionSortNet (bucketed causal sort-net scores).

Math (per bh slice; n=8192, bucket=64, nb=128 buckets, d=64):
  sq[i]  = cumavg(q)[64*i] / 8    = c8[i] * (q[64i] + sum_{j<i} qb[j])
  sk[j]  = H[j] * sum_{j'<j} kb[j'] + sum_s G[j,s] k[64j+s]
  R[i,jj] = sq[i] . skp[jj] ; skp = [0, sk[0..126]] padded front
  masked softmax over jj<=i, then keep strictly jj<i.

v3: all constants (G, H, c8, prefix matrices, masks, identities) are
generated ON DEVICE at start (gpsimd iota/affine_select + small PE
matmuls) - no const DMA traffic, so HBM transfers are inputs+outputs
only. The causal -inf mask is preloaded into the score PSUM bank and the
score matmul accumulates onto it (start=False), removing the mask add
from the critical tail. The q side computes raw bucket sums qb via 15
accumulating 256-col matmuls (identity stationary) plus a short DVE
strided reduce for the final 256-col chunk, then one matmul
qb^T(stat) x pmq(moving) produces the c8-weighted causal prefix already
TRANSPOSED ([d, i]) for the fp16 score matmul - the post-last-chunk
chain is reduce/add -> matmul -> add(f16) -> matmul -> exp -> mul -> DMA.
"""

import numpy as np
from contextlib import ExitStack

import concourse.bass as bass
import concourse.tile as tile
from concourse import mybir
from concourse import bass_utils

# ---------------- problem constants (hardcoded per spec) ----------------
BH_TOTAL = 32
N_CORES = 8
BH = BH_TOTAL // N_CORES          # 4 bh slices per core
SEQ = 8192
D = 64
BUCKET = 64
NB = SEQ // BUCKET                # 128 buckets
NJ = NB + 1                       # 129 output cols
NEG = -1e30

_F32 = mybir.dt.float32
_F32R = mybir.dt.float32r
_F16 = mybir.dt.float16
_I32 = mybir.dt.int32

KCH = 1024                        # k DMA / gpsimd chunk (cols)
MM = 512                          # k matmul moving width (one PSUM bank)
QCH = (2048, 1024, 512, 256, 256)  # q DMA chunks (cols)
QF = 16                           # 256-col fold matmuls into psQf


def _build_program(split_waits=True):
    nc = bass.Bass("TRN2", target_bir_lowering=False, debug=False)

    q_t = nc.dram_tensor("q", [BH, SEQ, D], _F32, kind="ExternalInput")
    k_t = nc.dram_tensor("k", [BH, SEQ, D], _F32, kind="ExternalInput")
    out_t = nc.dram_tensor("out", [BH, NB, NJ], _F32, kind="ExternalOutput")

    with tile.TileContext(nc) as tc, ExitStack() as ctx:
        _body(ctx, tc, q_t.ap(), k_t.ap(), out_t.ap())
    _fix_prep_lane_sems(nc)
    if split_waits:
        _split_matmul_waits(nc)
    return nc


def _fix_prep_lane_sems(nc):
    """Tile's wait pass emits consumer/epilogue waits against the round-robin
    DMASW lane semaphores, but the completion increment SDMA actually fires
    is the prep's own `sem=` (on_update[0]), which nothing rewires - leaving
    the DMASW waits unsatisfiable. Each lane here carries exactly one prep,
    so point every DMASW-lane wait at that prep's completion sem instead."""
    lane_sem = {}
    k = 0
    for f in nc.m.functions:
        for b in f.blocks:
            for i in b.instructions:
                if type(i).__name__ in ("InstKVWritebackAnt",
                                        "InstPagedWritebackAnt",
                                        "InstDMAGatherAnt",
                                        "InstDMAScatterAddAnt") \
                        and getattr(i, "gen_mode", 0) == 1:
                    u0 = i.sync_info.on_update[0]
                    lane = f"DMASW{k % 8}"
                    assert lane not in lane_sem, "one prep per lane assumed"
                    lane_sem[lane] = (u0.id, u0.ant_name)
                    k += 1
    n = 0
    for f in nc.m.functions:
        for b in f.blocks:
            for i in b.instructions:
                si = getattr(i, "sync_info", None)
                if not si or not si.on_wait:
                    continue
                waits = list(si.on_wait)
                changed = False
                for wi, w in enumerate(waits):
                    name = (getattr(w, "ant_name", "") or "").split("_")[0]
                    if name in lane_sem:
                        sid, sname = lane_sem[name]
                        waits[wi] = mybir.SyncWait(
                            sync_type=w.sync_type, id=sid, ant_name=sname,
                            wait_mode=w.wait_mode, wait_value=w.wait_value,
                            wait_reg=None)
                        changed = True
                        n += 1
                if changed:
                    i.sync_info = mybir.SyncInfo(on_wait=waits,
                                                 on_update=list(si.on_update))
    return n


_NO_SPLIT = ()


def _split_matmul_waits(nc):
    """This walrus build rejects compute instructions carrying more than one
    sync wait. Moving the waits onto single-wait NoOps placed immediately
    before the instruction in the same engine queue is semantically
    identical: the sequencer executes waits in queue order before
    dispatching."""
    n = 0
    for f in nc.m.functions:
        for b in f.blocks:
            insts = list(b.instructions)
            out = []
            changed = False
            for i in insts:
                si = getattr(i, "sync_info", None)
                if (si is not None and len(si.on_wait) > 1
                        and type(i).__name__ not in _NO_SPLIT
                        and i.is_executable()):
                    n += 1
                    changed = True
                    for wi, w in enumerate(si.on_wait):
                        nop = mybir.InstNoOp(
                            name=f"{i.name}-wsplit{wi}", ins=[], outs=[])
                        nop.engine = i.engine
                        nop.sync_info = mybir.SyncInfo(on_wait=[w], on_update=[])
                        out.append(nop)
                    i.sync_info = mybir.SyncInfo(
                        on_wait=[], on_update=list(si.on_update))
                out.append(i)
            if changed:
                b.instructions = out
    return n


def _gen_consts(nc, cpool, ppool):
    """Generate all constants on device.

    invT[s, j] = 1/(64j + s + 1)  ->  H row + suffix-sum GT via PE matmuls
    c8row[i] = 1/(512 i + 8); pmq/pmk = causal-masked outer products.
    """
    AL = mybir.AluOpType

    # integer iotas (gpsimd is the only engine with iota/affine_select)
    it_inv = cpool.tile([64, 128], _I32, tag="it_inv")
    nc.gpsimd.iota(it_inv[:], pattern=[[64, 128]], base=1, channel_multiplier=1)
    it_c8 = cpool.tile([1, 128], _I32, tag="it_c8")
    nc.gpsimd.iota(it_c8[:], pattern=[[512, 128]], base=8, channel_multiplier=0)

    # float conversions + reciprocals (DVE)
    invTf = cpool.tile([64, 128], _F32, tag="invTf")
    nc.vector.tensor_copy(invTf[:], it_inv[:])
    invTs = cpool.tile([64, 128], _F32, tag="invTs")
    nc.vector.reciprocal(invTs[:], invTf[:])
    invT = cpool.tile([64, 128], _F32R, tag="invT")
    nc.vector.tensor_copy(invT[:], invTs[:])
    c8f = cpool.tile([1, 128], _F32, tag="c8f")
    nc.vector.tensor_copy(c8f[:], it_c8[:])
    c8rowf = cpool.tile([1, 128], _F32, tag="c8rowf")
    nc.vector.reciprocal(c8rowf[:], c8f[:])
    c8row = cpool.tile([1, 128], _F32R, tag="c8row")
    nc.vector.tensor_copy(c8row[:], c8rowf[:])

    # ones / triangular / identity
    onesf = cpool.tile([64, 1], _F32, tag="onesf")
    nc.vector.memset(onesf[:], 1.0)
    ones64 = cpool.tile([64, 1], _F32R, tag="ones64")
    nc.vector.tensor_copy(ones64[:], onesf[:])
    onesf1 = cpool.tile([1, 128], _F32, tag="onesf1")
    nc.vector.memset(onesf1[:], 1.0)
    ones1 = cpool.tile([1, 128], _F32R, tag="ones1")
    nc.vector.tensor_copy(ones1[:], onesf1[:])
    U64f = cpool.tile([64, 64], _F32, tag="U64f")        # U[s',s]=1 iff s'>=s
    nc.gpsimd.memset(U64f[:], 1.0)
    nc.gpsimd.affine_select(out=U64f[:], in_=U64f[:], compare_op=AL.is_ge,
                            fill=0.0, base=0, pattern=[[-1, 64]],
                            channel_multiplier=1)
    U64 = cpool.tile([64, 64], _F32R, tag="U64")
    nc.vector.tensor_copy(U64[:], U64f[:])
    ident = cpool.tile([128, 128], _F32, tag="ident")    # f32, for transposes
    nc.gpsimd.memset(ident[:], 0.0)
    nc.gpsimd.affine_select(out=ident[:], in_=ident[:], compare_op=AL.not_equal,
                            fill=1.0, base=0, pattern=[[-1, 128]],
                            channel_multiplier=1)
    identr = cpool.tile([128, 128], _F32R, tag="identr")  # f32r, stationary
    nc.vector.tensor_copy(identr[:], ident[:])

    masknegf = cpool.tile([128, NJ + 3], _F32, tag="masknegf")
    nc.gpsimd.memset(masknegf[:], 0.0)
    nc.gpsimd.affine_select(out=masknegf[:], in_=masknegf[:],
                            compare_op=AL.is_ge, fill=NEG, base=0,
                            pattern=[[-1, NJ + 3]], channel_multiplier=1)
    maskneg = cpool.tile([128, NJ + 3], _F32R, tag="maskneg")
    nc.vector.tensor_copy(maskneg[:], masknegf[:])
    maskstrict = cpool.tile([128, NJ], _F32, tag="maskstrict")
    nc.gpsimd.memset(maskstrict[:], 1.0)
    nc.gpsimd.affine_select(out=maskstrict[:], in_=maskstrict[:],
                            compare_op=AL.is_ge, fill=0.0, base=-1,
                            pattern=[[-1, NJ]], channel_multiplier=1)

    # PE-derived rows: H = col-sums of invT; GT = suffix sums of invT.
    # Scratch PSUM borrows the loop's psK/psW tags (it is done long before
    # the first k matmuls) so psK/psW can double-buffer within 8 banks.
    psC0 = ppool.tile([128, MM], _F32, tag="psK", bufs=2)
    psC1 = ppool.tile([128, MM], _F32, tag="psW", bufs=2)
    nc.tensor.matmul(psC0[0:1, 0:128], ones64[:], invT[:],
                     start=True, stop=True)
    Hrowf = cpool.tile([1, 128], _F32, tag="Hrowf")
    nc.vector.tensor_copy(Hrowf[:], psC0[0:1, 0:128])
    Hrow = cpool.tile([1, 128], _F32R, tag="Hrow")
    nc.vector.tensor_copy(Hrow[:], Hrowf[:])
    nc.tensor.matmul(psC1[0:64, 0:128], U64[:], invT[:],
                     start=True, stop=True)
    gtS = cpool.tile([64, 128], _F32, tag="gtS")
    nc.scalar.copy(gtS[:], psC1[0:64, 0:128])

    # pmq[j,i] = (j<i) c8[i]; pmk[j,i] = (j<i) H[i]   (outer product + mask)
    nc.tensor.matmul(psC0[:, 0:128], ones1[:], c8row[:],
                     start=True, stop=True)
    pmqf = cpool.tile([128, 128], _F32, tag="pmqf")
    nc.scalar.copy(pmqf[:], psC0[:, 0:128])
    nc.gpsimd.affine_select(out=pmqf[:], in_=pmqf[:], compare_op=AL.is_ge,
                            fill=0.0, base=-1, pattern=[[1, 128]],
                            channel_multiplier=-1)
    pmq = cpool.tile([128, 128], _F16, tag="pmq")        # moving operand
    nc.vector.tensor_copy(pmq[:], pmqf[:])
    nc.tensor.matmul(psC1[:, 0:128], ones1[:], Hrow[:],
                     start=True, stop=True)
    pmkf = cpool.tile([128, 128], _F32, tag="pmkf")
    nc.scalar.copy(pmkf[:], psC1[:, 0:128])
    nc.gpsimd.affine_select(out=pmkf[:], in_=pmkf[:], compare_op=AL.is_ge,
                            fill=0.0, base=-1, pattern=[[1, 128]],
                            channel_multiplier=-1)
    pmk = cpool.tile([128, 128], _F32R, tag="pmk")       # stationary
    nc.vector.tensor_copy(pmk[:], pmkf[:])

    # G[j, s] = transpose(GT)
    nc.tensor.transpose(psC0[:, 0:64], gtS[:], ident[0:64, 0:64])
    gs = cpool.tile([128, 64], _F32, tag="gs")
    nc.vector.tensor_copy(gs[:], psC0[:, 0:64])

    # c8 broadcast along 64 partitions for the q0T scale (outer product)
    nc.tensor.matmul(psC1[0:64, 0:128], ones1[:, 0:64], c8row[:],
                     start=True, stop=True)
    c8bT = cpool.tile([64, 128], _F32, tag="c8bT")
    nc.vector.tensor_copy(c8bT[:], psC1[0:64, 0:128])

    return dict(gs=gs, ident=ident, identr=identr, pmq=pmq, pmk=pmk,
                c8bT=c8bT, maskneg=maskneg, maskstrict=maskstrict)


def _body(ctx, tc, q, k, out):
    nc = tc.nc
    cpool = ctx.enter_context(tc.tile_pool(name="consts", bufs=1))
    dpool = ctx.enter_context(tc.tile_pool(name="data", bufs=2))
    spool = ctx.enter_context(tc.tile_pool(name="small", bufs=2))
    ppool = ctx.enter_context(tc.tile_pool(name="psum", bufs=2, space="PSUM"))

    C = _gen_consts(nc, cpool, ppool)
    gs, ident, identr = C["gs"], C["ident"], C["identr"]
    pmq, pmk, c8bT = C["pmq"], C["pmk"], C["c8bT"]
    maskneg, maskstrict = C["maskneg"], C["maskstrict"]

    def vds(t, s):
        # [128, s*64] -> [128, d, s] view for the strided s-reduce
        return t.rearrange("j (s d) -> j d s", s=s, d=64)

    outbs = []
    for bh in range(BH):
        # ---- k loads (4 chunks), G-multiplies, PE prefix/fold matmuls ----
        kt = dpool.tile([128, 4096], _F32, tag="kt", bufs=2)
        ksrc = k[bh].rearrange("(j r) d -> j (r d)", r=64)
        for c in range(4):
            sl = slice(KCH * c, KCH * (c + 1))
            nc.sync.dma_start(kt[:, sl].bitcast(_F32R),
                              ksrc[:, sl].bitcast(_F32R))

        # kw = kt * G (d-broadcast): all chunks on DVE (faster per element
        # than GPSIMD), keeping the Pool queue free for the SWDGE output
        # preps/triggers - a Pool kw multiply queued behind a prep that
        # waits on the previous slice's softmax couples the k pipeline to
        # the previous tail through Pool's in-order engine ticks.
        kw = dpool.tile([128, 4096], _F32R, tag="kw", bufs=2)
        for c, eng in ((0, nc.vector), (1, nc.vector), (2, nc.vector),
                       (3, nc.vector)):
            sl = slice(KCH * c, KCH * (c + 1))
            gb = gs[:, 16 * c:16 * (c + 1)].unsqueeze(2).broadcast_to(
                [128, 16, 64])
            eng.tensor_mul(
                kw[:, sl].rearrange("j (s d) -> j s d", d=64),
                kt[:, sl].rearrange("j (s d) -> j s d", d=64),
                gb)

        # PE: k prefix (pmk) and kw-sum (identity) matmuls, emitted in data-
        # readiness order (kt chunk c arrives before mult c completes)
        psK = ppool.tile([128, MM], _F32, tag="psK", bufs=2)
        psW = ppool.tile([128, MM], _F32, tag="psW", bufs=2)

        def mmK(c):
            nc.tensor.matmul(psK[:], pmk[:], kt[:, MM * c:MM * (c + 1)].bitcast(_F32R),
                             start=(c == 0), stop=(c == 7))

        def mmW(c):
            nc.tensor.matmul(psW[:], identr[:], kw[:, MM * c:MM * (c + 1)],
                             start=(c == 0), stop=(c == 7))

        mmK(0); mmK(1); mmK(2); mmK(3)
        mmW(0); mmW(1)
        mmK(4); mmK(5)
        mmW(2); mmW(3)
        mmK(6); mmK(7)
        mmW(4); mmW(5); mmW(6); mmW(7)

        # ---- early per-bh prep (no data deps) ----
        psS = ppool.tile([128, MM], _F32, tag="psS", bufs=1)
        # causal -inf mask preloaded via a PE copy-matmul (identity
        # stationary x maskneg moving) that OPENS the psS accumulation
        # group; the tail's score matmul then accumulates onto it with
        # start=False. A DVE write into PSUM is not reliably visible to
        # the PE accumulation path on hardware.
        nc.tensor.matmul(psS[:, 0:NJ + 3], identr[:], maskneg[:],
                         start=True, stop=False)
        skpT = spool.tile([64, NJ + 3], _F16, tag="skpT")
        nc.vector.memset(skpT[:], 0.0)
        outb = spool.tile([128, NJ], _F32, tag="outb", bufs=4)
        outbs.append(outb)


        # ---- k-side combine (DVE) ----
        kpre = spool.tile([128, D], _F32, tag="kpre")
        nc.vector.reduce_sum(kpre[:], vds(psK[:], 8), axis=mybir.AxisListType.X)
        kg = spool.tile([128, D], _F32, tag="kg")
        nc.vector.reduce_sum(kg[:], vds(psW[:], 8), axis=mybir.AxisListType.X)
        sk = spool.tile([128, D], _F32, tag="sk")
        nc.vector.tensor_add(sk[:], kg[:], kpre[:])

        psT = ppool.tile([64, 256], _F32, tag="psT", bufs=1)
        nc.tensor.transpose(psT[0:64, 0:128], sk[:], ident[:])
        nc.scalar.copy(skpT[:, 1:NJ], psT[0:64, 0:128])

        # ---- q loads ----
        qt = dpool.tile([128, 4096], _F32, tag="qt", bufs=2)
        qsrc = q[bh].rearrange("(j r) d -> j (r d)", r=64)
        o = 0
        for ln in QCH:
            nc.sync.dma_start(qt[:, o:o + ln].bitcast(_F32R),
                              qsrc[:, o:o + ln].bitcast(_F32R))
            o += ln

        # ---- q bucket sums: psQf accumulates cols 0:3840 (15 x 256) ----
        psQf = ppool.tile([128, 256], _F32, tag="psQf", bufs=1)
        for m in range(8):
            nc.tensor.matmul(psQf[:], identr[:],
                             qt[:, 256 * m:256 * (m + 1)].bitcast(_F32R),
                             start=(m == 0), stop=False)
        # q0T while chunk 0 is resident; c8-scaled on DVE (off tail)
        nc.tensor.transpose(psT[0:64, 128:256], qt[:, 0:D], ident[0:128, :])
        c8q0T = spool.tile([64, 128], _F32, tag="c8q0T")
        nc.vector.tensor_mul(c8q0T[:], psT[0:64, 128:256], c8bT[:])
        for m in range(8, QF):
            nc.tensor.matmul(psQf[:], identr[:],
                             qt[:, 256 * m:256 * (m + 1)].bitcast(_F32R),
                             start=False, stop=(m == QF - 1))

        # ---- tail: close fold, prefix matmul, scores, softmax ----
        qb = spool.tile([128, D], _F16, tag="qb")
        with nc.allow_low_precision(reason="qb feeds an f16 score matmul"):
            nc.vector.reduce_sum(qb[:], vds(psQf[:], 4),
                                 axis=mybir.AxisListType.X)
        psBQT = ppool.tile([64, 128], _F32, tag="psBQT", bufs=1)
        nc.tensor.matmul(psBQT[:], qb[:], pmq[:],
                         start=True, stop=True)
        sqT16 = spool.tile([64, 128], _F16, tag="sqT16")
        nc.vector.tensor_add(sqT16[:], psBQT[:], c8q0T[:])
        nc.tensor.matmul(psS[:, 0:NJ + 3], sqT16[:], skpT[:],
                         start=False, stop=True)

        e = spool.tile([128, NJ], _F32, tag="e")
        den = spool.tile([128, 1], _F32, tag="den")
        nc.scalar.activation(e[:], psS[:, 0:NJ],
                             mybir.ActivationFunctionType.Exp,
                             bias=0.0, scale=1.0, accum_out=den[:])
        rden = spool.tile([128, 1], _F32, tag="rden")
        nc.vector.reciprocal(rden[:], den[:])
        nc.vector.scalar_tensor_tensor(outb[:], e[:], rden[:], maskstrict[:],
                                       op0=mybir.AluOpType.mult,
                                       op1=mybir.AluOpType.mult)

    # All output DMAs issue from SP AFTER the whole input stream: their
    # transfers land in the tail's natural DMA idle. Issuing them
    # mid-stream (e.g. from Act) parks them for microseconds behind the
    # saturated DMA engines, and the 8 round-robin HWDGE completion lanes
    # then make an input DMA eight slots later wait on the parked output.
    for bh, outb in enumerate(outbs):
        nc.sync.dma_start(out[bh], outb[:])


_CACHE = {}


def _get_program():
    if "nc" not in _CACHE:
        _CACHE["nc"] = _build_program()
        _CACHE["consts"] = {}
    return _CACHE["nc"], _CACHE["consts"]


def _get_runner():
    """Build the sharded PJRT callable once and cache it (mirrors
    bass2jax.run_bass_via_pjrt but reuses the jitted function across
    calls)."""
    if "runner" in _CACHE:
        return _CACHE["runner"]
    import jax
    from jax.sharding import Mesh, PartitionSpec
    from jax.experimental.shard_map import shard_map
    from concourse import bass2jax

    nc, consts = _get_program()
    bass2jax.install_neuronx_cc_hook()

    part_name = nc.partition_id_tensor.name if nc.partition_id_tensor else None
    in_names, out_names, out_avals, zero_outs = [], [], [], []
    for alloc in nc.m.functions[0].allocations:
        if not isinstance(alloc, mybir.MemoryLocationSet):
            continue
        name = alloc.memorylocations[0].name
        if alloc.kind == "ExternalInput":
            if name != part_name:
                in_names.append(name)
        elif alloc.kind == "ExternalOutput":
            out_names.append(name)
            shape = tuple(alloc.tensor_shape)
            dtype = mybir.dt.np(alloc.dtype)
            out_avals.append(jax.core.ShapedArray(shape, dtype))
            zero_outs.append(np.zeros(shape, dtype))
    n_params = len(in_names)
    all_names = in_names + out_names
    if part_name is not None:
        all_names = all_names + [part_name]
    donate = tuple(range(n_params, n_params + len(out_names)))

    def _body(*args):
        operands = list(args)
        if part_name is not None:
            operands.append(bass2jax.partition_id_tensor())
        outs = bass2jax._bass_exec_p.bind(
            *operands,
            out_avals=tuple(out_avals),
            in_names=tuple(all_names),
            out_names=tuple(out_names),
            lowering_input_output_aliases=(),
            sim_require_finite=True,
            sim_require_nnan=True,
            nc=nc,
        )
        return tuple(outs)

    devices = jax.devices()[:N_CORES]
    mesh = Mesh(np.asarray(devices), ("core",))
    specs = (PartitionSpec("core"),) * (n_params + len(out_names))
    sharded = jax.jit(
        shard_map(_body, mesh=mesh, in_specs=specs,
                  out_specs=(PartitionSpec("core"),) * len(out_names),
                  check_rep=False),
        donate_argnums=donate, keep_unused=True,
    )
    runner = dict(fn=sharded, in_names=in_names, out_names=out_names,
                  zero_outs=zero_outs, consts=consts, nc=nc)
    _CACHE["runner"] = runner
    return runner


def _concat_inputs(q, k, runner):
    """Per-core input dict -> globally concatenated arrays (axis 0)."""
    arrs = []
    for name in runner["in_names"]:
        if name == "q":
            arrs.append(q)
        elif name == "k":
            arrs.append(k)
        else:
            raise KeyError(name)
    return arrs


def kernel(q, k):
    q = np.ascontiguousarray(np.asarray(q, dtype=np.float32))
    k = np.ascontiguousarray(np.asarray(k, dtype=np.float32))
    assert q.shape == (BH_TOTAL, SEQ, D) and k.shape == (BH_TOTAL, SEQ, D)

    runner = _get_runner()
    # bh-shard across 8 cores: core c gets bh slice [4c, 4c+4). The global
    # concat layout [32, ...] already matches (shard_map splits axis 0).
    concat_in = _concat_inputs(q, k, runner)
    concat_zeros = [np.zeros((N_CORES * z.shape[0], *z.shape[1:]), z.dtype)
                    for z in runner["zero_outs"]]
    out_arrs = runner["fn"](*concat_in, *concat_zeros)
    out = np.asarray(out_arrs[0])          # [8*4, 128, 129]
    return np.ascontiguousarray(out.reshape(BH_TOTAL, NB, NJ))


# revision 43
# speedup vs baseline: 1.0617x; 1.0030x over previous
"""Trainium2 Bass kernel for CausalAttentionSortNet (bucketed causal sort-net scores).

Math (per bh slice; n=8192, bucket=64, nb=128 buckets, d=64):
  sq[i]  = cumavg(q)[64*i] / 8    = c8[i] * (q[64i] + sum_{j<i} qb[j])
  sk[j]  = H[j] * sum_{j'<j} kb[j'] + sum_s G[j,s] k[64j+s]
  R[i,jj] = sq[i] . skp[jj] ; skp = [0, sk[0..126]] padded front
  masked softmax over jj<=i, then keep strictly jj<i.

v3: all constants (G, H, c8, prefix matrices, masks, identities) are
generated ON DEVICE at start (gpsimd iota/affine_select + small PE
matmuls) - no const DMA traffic, so HBM transfers are inputs+outputs
only. The causal -inf mask is preloaded into the score PSUM bank and the
score matmul accumulates onto it (start=False), removing the mask add
from the critical tail. The q side computes raw bucket sums qb via 15
accumulating 256-col matmuls (identity stationary) plus a short DVE
strided reduce for the final 256-col chunk, then one matmul
qb^T(stat) x pmq(moving) produces the c8-weighted causal prefix already
TRANSPOSED ([d, i]) for the fp16 score matmul - the post-last-chunk
chain is reduce/add -> matmul -> add(f16) -> matmul -> exp -> mul -> DMA.
"""

import numpy as np
from contextlib import ExitStack

import concourse.bass as bass
import concourse.tile as tile
from concourse import mybir
from concourse import bass_utils

# ---------------- problem constants (hardcoded per spec) ----------------
BH_TOTAL = 32
N_CORES = 8
BH = BH_TOTAL // N_CORES          # 4 bh slices per core
SEQ = 8192
D = 64
BUCKET = 64
NB = SEQ // BUCKET                # 128 buckets
NJ = NB + 1                       # 129 output cols
NEG = -1e30

_F32 = mybir.dt.float32
_F32R = mybir.dt.float32r
_F16 = mybir.dt.float16
_I32 = mybir.dt.int32

KCH = 1024                        # k DMA / gpsimd chunk (cols)
MM = 512                          # k matmul moving width (one PSUM bank)
QCH = (2048, 1024, 512, 256, 256)  # q DMA chunks (cols)
QF = 16                           # 256-col fold matmuls into psQf


def _build_program(split_waits=True):
    nc = bass.Bass("TRN2", target_bir_lowering=False, debug=False)

    q_t = nc.dram_tensor("q", [BH, SEQ, D], _F32, kind="ExternalInput")
    k_t = nc.dram_tensor("k", [BH, SEQ, D], _F32, kind="ExternalInput")
    out_t = nc.dram_tensor("out", [BH, NB, NJ], _F32, kind="ExternalOutput")

    with tile.TileContext(nc) as tc, ExitStack() as ctx:
        _body(ctx, tc, q_t.ap(), k_t.ap(), out_t.ap())
    _fix_prep_lane_sems(nc)
    if split_waits:
        _split_matmul_waits(nc)
    return nc


def _fix_prep_lane_sems(nc):
    """Tile's wait pass emits consumer/epilogue waits against the round-robin
    DMASW lane semaphores, but the completion increment SDMA actually fires
    is the prep's own `sem=` (on_update[0]), which nothing rewires - leaving
    the DMASW waits unsatisfiable. Each lane here carries exactly one prep,
    so point every DMASW-lane wait at that prep's completion sem instead."""
    lane_sem = {}
    k = 0
    for f in nc.m.functions:
        for b in f.blocks:
            for i in b.instructions:
                if type(i).__name__ in ("InstKVWritebackAnt",
                                        "InstPagedWritebackAnt",
                                        "InstDMAGatherAnt",
                                        "InstDMAScatterAddAnt") \
                        and getattr(i, "gen_mode", 0) == 1:
                    u0 = i.sync_info.on_update[0]
                    lane = f"DMASW{k % 8}"
                    assert lane not in lane_sem, "one prep per lane assumed"
                    lane_sem[lane] = (u0.id, u0.ant_name)
                    k += 1
    n = 0
    for f in nc.m.functions:
        for b in f.blocks:
            for i in b.instructions:
                si = getattr(i, "sync_info", None)
                if not si or not si.on_wait:
                    continue
                waits = list(si.on_wait)
                changed = False
                for wi, w in enumerate(waits):
                    name = (getattr(w, "ant_name", "") or "").split("_")[0]
                    if name in lane_sem:
                        sid, sname = lane_sem[name]
                        waits[wi] = mybir.SyncWait(
                            sync_type=w.sync_type, id=sid, ant_name=sname,
                            wait_mode=w.wait_mode, wait_value=w.wait_value,
                            wait_reg=None)
                        changed = True
                        n += 1
                if changed:
                    i.sync_info = mybir.SyncInfo(on_wait=waits,
                                                 on_update=list(si.on_update))
    return n


_NO_SPLIT = ()


def _split_matmul_waits(nc):
    """This walrus build rejects compute instructions carrying more than one
    sync wait. Moving the waits onto single-wait NoOps placed immediately
    before the instruction in the same engine queue is semantically
    identical: the sequencer executes waits in queue order before
    dispatching."""
    n = 0
    for f in nc.m.functions:
        for b in f.blocks:
            insts = list(b.instructions)
            out = []
            changed = False
            for i in insts:
                si = getattr(i, "sync_info", None)
                if (si is not None and len(si.on_wait) > 1
                        and type(i).__name__ not in _NO_SPLIT
                        and i.is_executable()):
                    n += 1
                    changed = True
                    for wi, w in enumerate(si.on_wait):
                        nop = mybir.InstNoOp(
                            name=f"{i.name}-wsplit{wi}", ins=[], outs=[])
                        nop.engine = i.engine
                        nop.sync_info = mybir.SyncInfo(on_wait=[w], on_update=[])
                        out.append(nop)
                    i.sync_info = mybir.SyncInfo(
                        on_wait=[], on_update=list(si.on_update))
                out.append(i)
            if changed:
                b.instructions = out
    return n


def _gen_consts(nc, cpool, ppool):
    """Generate all constants on device.

    invT[s, j] = 1/(64j + s + 1)  ->  H row + suffix-sum GT via PE matmuls
    c8row[i] = 1/(512 i + 8); pmq/pmk = causal-masked outer products.
    """
    AL = mybir.AluOpType

    # integer iotas (gpsimd is the only engine with iota/affine_select)
    it_inv = cpool.tile([64, 128], _I32, tag="it_inv")
    nc.gpsimd.iota(it_inv[:], pattern=[[64, 128]], base=1, channel_multiplier=1)
    it_c8 = cpool.tile([1, 128], _I32, tag="it_c8")
    nc.gpsimd.iota(it_c8[:], pattern=[[512, 128]], base=8, channel_multiplier=0)

    # float conversions + reciprocals (DVE)
    invTf = cpool.tile([64, 128], _F32, tag="invTf")
    nc.vector.tensor_copy(invTf[:], it_inv[:])
    invTs = cpool.tile([64, 128], _F32, tag="invTs")
    nc.vector.reciprocal(invTs[:], invTf[:])
    invT = cpool.tile([64, 128], _F32R, tag="invT")
    nc.vector.tensor_copy(invT[:], invTs[:])
    c8f = cpool.tile([1, 128], _F32, tag="c8f")
    nc.vector.tensor_copy(c8f[:], it_c8[:])
    c8rowf = cpool.tile([1, 128], _F32, tag="c8rowf")
    nc.vector.reciprocal(c8rowf[:], c8f[:])
    c8row = cpool.tile([1, 128], _F32R, tag="c8row")
    nc.vector.tensor_copy(c8row[:], c8rowf[:])

    # ones / triangular / identity
    onesf = cpool.tile([64, 1], _F32, tag="onesf")
    nc.vector.memset(onesf[:], 1.0)
    ones64 = cpool.tile([64, 1], _F32R, tag="ones64")
    nc.vector.tensor_copy(ones64[:], onesf[:])
    onesf1 = cpool.tile([1, 128], _F32, tag="onesf1")
    nc.vector.memset(onesf1[:], 1.0)
    ones1 = cpool.tile([1, 128], _F32R, tag="ones1")
    nc.vector.tensor_copy(ones1[:], onesf1[:])
    U64f = cpool.tile([64, 64], _F32, tag="U64f")        # U[s',s]=1 iff s'>=s
    nc.gpsimd.memset(U64f[:], 1.0)
    nc.gpsimd.affine_select(out=U64f[:], in_=U64f[:], compare_op=AL.is_ge,
                            fill=0.0, base=0, pattern=[[-1, 64]],
                            channel_multiplier=1)
    U64 = cpool.tile([64, 64], _F32R, tag="U64")
    nc.vector.tensor_copy(U64[:], U64f[:])
    ident = cpool.tile([128, 128], _F32, tag="ident")    # f32, for transposes
    nc.gpsimd.memset(ident[:], 0.0)
    nc.gpsimd.affine_select(out=ident[:], in_=ident[:], compare_op=AL.not_equal,
                            fill=1.0, base=0, pattern=[[-1, 128]],
                            channel_multiplier=1)
    identr = cpool.tile([128, 128], _F32R, tag="identr")  # f32r, stationary
    nc.vector.tensor_copy(identr[:], ident[:])
    ident16 = cpool.tile([128, 64], _F16, tag="ident16")  # f16, stationary
    nc.vector.tensor_copy(ident16[:], ident[:, 0:64])

    masknegf = cpool.tile([128, NJ + 3], _F32, tag="masknegf")
    nc.gpsimd.memset(masknegf[:], 0.0)
    nc.gpsimd.affine_select(out=masknegf[:], in_=masknegf[:],
                            compare_op=AL.is_ge, fill=NEG, base=0,
                            pattern=[[-1, NJ + 3]], channel_multiplier=1)
    maskneg = cpool.tile([128, NJ + 3], _F32R, tag="maskneg")
    nc.vector.tensor_copy(maskneg[:], masknegf[:])
    maskstrict = cpool.tile([128, NJ], _F32, tag="maskstrict")
    nc.gpsimd.memset(maskstrict[:], 1.0)
    nc.gpsimd.affine_select(out=maskstrict[:], in_=maskstrict[:],
                            compare_op=AL.is_ge, fill=0.0, base=-1,
                            pattern=[[-1, NJ]], channel_multiplier=1)

    # PE-derived rows: H = col-sums of invT; GT = suffix sums of invT.
    # Scratch PSUM borrows the loop's psK/psW tags (it is done long before
    # the first k matmuls) so psK/psW can double-buffer within 8 banks.
    psC0 = ppool.tile([128, MM], _F32, tag="psK", bufs=2)
    psC1 = ppool.tile([128, MM], _F32, tag="psW", bufs=2)
    nc.tensor.matmul(psC0[0:1, 0:128], ones64[:], invT[:],
                     start=True, stop=True)
    Hrowf = cpool.tile([1, 128], _F32, tag="Hrowf")
    nc.vector.tensor_copy(Hrowf[:], psC0[0:1, 0:128])
    Hrow = cpool.tile([1, 128], _F32R, tag="Hrow")
    nc.vector.tensor_copy(Hrow[:], Hrowf[:])
    nc.tensor.matmul(psC1[0:64, 0:128], U64[:], invT[:],
                     start=True, stop=True)
    gtS = cpool.tile([64, 128], _F32, tag="gtS")
    nc.scalar.copy(gtS[:], psC1[0:64, 0:128])

    # pmq[j,i] = (j<i) c8[i]; pmk[j,i] = (j<i) H[i]   (outer product + mask)
    nc.tensor.matmul(psC0[:, 0:128], ones1[:], c8row[:],
                     start=True, stop=True)
    pmqf = cpool.tile([128, 128], _F32, tag="pmqf")
    nc.scalar.copy(pmqf[:], psC0[:, 0:128])
    nc.gpsimd.affine_select(out=pmqf[:], in_=pmqf[:], compare_op=AL.is_ge,
                            fill=0.0, base=-1, pattern=[[1, 128]],
                            channel_multiplier=-1)
    pmq = cpool.tile([128, 128], _F16, tag="pmq")        # moving operand
    nc.vector.tensor_copy(pmq[:], pmqf[:])
    nc.tensor.matmul(psC1[:, 0:128], ones1[:], Hrow[:],
                     start=True, stop=True)
    pmkf = cpool.tile([128, 128], _F32, tag="pmkf")
    nc.scalar.copy(pmkf[:], psC1[:, 0:128])
    nc.gpsimd.affine_select(out=pmkf[:], in_=pmkf[:], compare_op=AL.is_ge,
                            fill=0.0, base=-1, pattern=[[1, 128]],
                            channel_multiplier=-1)
    pmk = cpool.tile([128, 128], _F32R, tag="pmk")       # stationary
    nc.vector.tensor_copy(pmk[:], pmkf[:])

    # G[j, s] = transpose(GT)
    nc.tensor.transpose(psC0[:, 0:64], gtS[:], ident[0:64, 0:64])
    gs = cpool.tile([128, 64], _F32, tag="gs")
    nc.vector.tensor_copy(gs[:], psC0[:, 0:64])

    # c8 broadcast along 64 partitions for the q0T scale (outer product)
    nc.tensor.matmul(psC1[0:64, 0:128], ones1[:, 0:64], c8row[:],
                     start=True, stop=True)
    c8bT = cpool.tile([64, 128], _F32, tag="c8bT")
    nc.vector.tensor_copy(c8bT[:], psC1[0:64, 0:128])

    return dict(gs=gs, ident=ident, identr=identr, pmq=pmq, pmk=pmk,
                c8bT=c8bT, maskneg=maskneg, maskstrict=maskstrict,
                ident16=ident16)


def _body(ctx, tc, q, k, out):
    nc = tc.nc
    cpool = ctx.enter_context(tc.tile_pool(name="consts", bufs=1))
    dpool = ctx.enter_context(tc.tile_pool(name="data", bufs=2))
    spool = ctx.enter_context(tc.tile_pool(name="small", bufs=2))
    ppool = ctx.enter_context(tc.tile_pool(name="psum", bufs=2, space="PSUM"))

    C = _gen_consts(nc, cpool, ppool)
    gs, ident, identr = C["gs"], C["ident"], C["identr"]
    pmq, pmk, c8bT = C["pmq"], C["pmk"], C["c8bT"]
    maskneg, maskstrict = C["maskneg"], C["maskstrict"]
    ident16 = C["ident16"]

    def vds(t, s):
        # [128, s*64] -> [128, d, s] view for the strided s-reduce
        return t.rearrange("j (s d) -> j d s", s=s, d=64)

    c8q0T = cpool.tile([128, 128], _F16, tag="c8q0T")
    nc.vector.memset(c8q0T[64:128, :], 0.0)

    outbs = []
    for bh in range(BH):
        # ---- k loads (4 chunks), G-multiplies, PE prefix/fold matmuls ----
        kt = dpool.tile([128, 4096], _F32, tag="kt", bufs=2)
        ksrc = k[bh].rearrange("(j r) d -> j (r d)", r=64)
        for c in range(4):
            sl = slice(KCH * c, KCH * (c + 1))
            nc.sync.dma_start(kt[:, sl].bitcast(_F32R),
                              ksrc[:, sl].bitcast(_F32R))

        # kw = kt * G (d-broadcast): all chunks on DVE (faster per element
        # than GPSIMD), keeping the Pool queue free for the SWDGE output
        # preps/triggers - a Pool kw multiply queued behind a prep that
        # waits on the previous slice's softmax couples the k pipeline to
        # the previous tail through Pool's in-order engine ticks.
        kw = dpool.tile([128, 4096], _F32R, tag="kw", bufs=2)
        for c, eng in ((0, nc.vector), (1, nc.vector), (2, nc.vector),
                       (3, nc.vector)):
            sl = slice(KCH * c, KCH * (c + 1))
            gb = gs[:, 16 * c:16 * (c + 1)].unsqueeze(2).broadcast_to(
                [128, 16, 64])
            eng.tensor_mul(
                kw[:, sl].rearrange("j (s d) -> j s d", d=64),
                kt[:, sl].rearrange("j (s d) -> j s d", d=64),
                gb)

        # PE: k prefix (pmk) and kw-sum (identity) matmuls, emitted in data-
        # readiness order (kt chunk c arrives before mult c completes)
        psK = ppool.tile([128, MM], _F32, tag="psK", bufs=2)
        psW = ppool.tile([128, MM], _F32, tag="psW", bufs=2)

        def mmK(c):
            nc.tensor.matmul(psK[:], pmk[:], kt[:, MM * c:MM * (c + 1)].bitcast(_F32R),
                             start=(c == 0), stop=(c == 7))

        def mmW(c):
            nc.tensor.matmul(psW[:], identr[:], kw[:, MM * c:MM * (c + 1)],
                             start=(c == 0), stop=(c == 7))

        mmK(0); mmK(1); mmK(2); mmK(3)
        mmW(0); mmW(1)
        mmK(4); mmK(5)
        mmW(2); mmW(3)
        mmK(6); mmK(7)
        mmW(4); mmW(5); mmW(6); mmW(7)

        # ---- early per-bh prep (no data deps) ----
        psS = ppool.tile([128, MM], _F32, tag="psS", bufs=1)
        # causal -inf mask preloaded via a PE copy-matmul (identity
        # stationary x maskneg moving) that OPENS the psS accumulation
        # group; the tail's score matmul then accumulates onto it with
        # start=False. A DVE write into PSUM is not reliably visible to
        # the PE accumulation path on hardware.
        nc.tensor.matmul(psS[:, 0:NJ + 3], identr[:], maskneg[:],
                         start=True, stop=False)
        skpT = spool.tile([64, NJ + 3], _F16, tag="skpT")
        nc.vector.memset(skpT[:], 0.0)
        outb = spool.tile([128, NJ], _F32, tag="outb", bufs=4)
        outbs.append(outb)


        # ---- k-side combine (DVE) ----
        kpre = spool.tile([128, D], _F32, tag="kpre")
        nc.vector.reduce_sum(kpre[:], vds(psK[:], 8), axis=mybir.AxisListType.X)
        kg = spool.tile([128, D], _F32, tag="kg")
        nc.vector.reduce_sum(kg[:], vds(psW[:], 8), axis=mybir.AxisListType.X)
        sk = spool.tile([128, D], _F32, tag="sk")
        nc.vector.tensor_add(sk[:], kg[:], kpre[:])

        psT = ppool.tile([64, 256], _F32, tag="psT", bufs=1)
        nc.tensor.transpose(psT[0:64, 0:128], sk[:], ident[:])
        nc.scalar.copy(skpT[:, 1:NJ], psT[0:64, 0:128])

        # ---- q loads ----
        qt = dpool.tile([128, 4096], _F32, tag="qt", bufs=2)
        qsrc = q[bh].rearrange("(j r) d -> j (r d)", r=64)
        o = 0
        for ln in QCH:
            nc.sync.dma_start(qt[:, o:o + ln].bitcast(_F32R),
                              qsrc[:, o:o + ln].bitcast(_F32R))
            o += ln

        # ---- q bucket sums: psQf accumulates cols 0:3840 (15 x 256) ----
        psQf = ppool.tile([128, 256], _F32, tag="psQf", bufs=1)
        for m in range(8):
            nc.tensor.matmul(psQf[:], identr[:],
                             qt[:, 256 * m:256 * (m + 1)].bitcast(_F32R),
                             start=(m == 0), stop=False)
        # q0T while chunk 0 is resident; c8-scaled on DVE (off tail)
        nc.tensor.transpose(psT[0:64, 128:256], qt[:, 0:D], ident[0:128, :])
        nc.vector.tensor_mul(c8q0T[0:64, :], psT[0:64, 128:256], c8bT[:])
        # open the psBQT accumulation group mid-stream with the c8*q0T term;
        # both group members contract over 128 partitions so the PE tile
        # config (128, 64) is identical across the group (mixing tile sizes
        # inside one accumulation group NaN'd on hardware).
        psBQT = ppool.tile([64, 128], _F32, tag="psBQT", bufs=1)
        nc.tensor.matmul(psBQT[:], ident16[:], c8q0T[:],
                         start=True, stop=False)
        for m in range(8, QF):
            nc.tensor.matmul(psQf[:], identr[:],
                             qt[:, 256 * m:256 * (m + 1)].bitcast(_F32R),
                             start=False, stop=(m == QF - 1))

        # ---- tail: close fold, prefix matmul, scores, softmax ----
        qb = spool.tile([128, D], _F16, tag="qb")
        with nc.allow_low_precision(reason="qb feeds an f16 score matmul"):
            nc.vector.reduce_sum(qb[:], vds(psQf[:], 4),
                                 axis=mybir.AxisListType.X)
        nc.tensor.matmul(psBQT[:], qb[:], pmq[:],
                         start=False, stop=True)
        sqT16 = spool.tile([64, 128], _F16, tag="sqT16")
        nc.vector.tensor_copy(sqT16[:], psBQT[:])
        nc.tensor.matmul(psS[:, 0:NJ + 3], sqT16[:], skpT[:],
                         start=False, stop=True)

        e = spool.tile([128, NJ], _F32, tag="e")
        den = spool.tile([128, 1], _F32, tag="den")
        nc.scalar.activation(e[:], psS[:, 0:NJ],
                             mybir.ActivationFunctionType.Exp,
                             bias=0.0, scale=1.0, accum_out=den[:])
        rden = spool.tile([128, 1], _F32, tag="rden")
        nc.vector.reciprocal(rden[:], den[:])
        nc.vector.scalar_tensor_tensor(outb[:], e[:], rden[:], maskstrict[:],
                                       op0=mybir.AluOpType.mult,
                                       op1=mybir.AluOpType.mult)

    # All output DMAs issue from SP AFTER the whole input stream: their
    # transfers land in the tail's natural DMA idle. Issuing them
    # mid-stream (e.g. from Act) parks them for microseconds behind the
    # saturated DMA engines, and the 8 round-robin HWDGE completion lanes
    # then make an input DMA eight slots later wait on the parked output.
    for bh, outb in enumerate(outbs):
        nc.sync.dma_start(out[bh], outb[:])


_CACHE = {}


def _get_program():
    if "nc" not in _CACHE:
        _CACHE["nc"] = _build_program()
        _CACHE["consts"] = {}
    return _CACHE["nc"], _CACHE["consts"]


def _get_runner():
    """Build the sharded PJRT callable once and cache it (mirrors
    bass2jax.run_bass_via_pjrt but reuses the jitted function across
    calls)."""
    if "runner" in _CACHE:
        return _CACHE["runner"]
    import jax
    from jax.sharding import Mesh, PartitionSpec
    from jax.experimental.shard_map import shard_map
    from concourse import bass2jax

    nc, consts = _get_program()
    bass2jax.install_neuronx_cc_hook()

    part_name = nc.partition_id_tensor.name if nc.partition_id_tensor else None
    in_names, out_names, out_avals, zero_outs = [], [], [], []
    for alloc in nc.m.functions[0].allocations:
        if not isinstance(alloc, mybir.MemoryLocationSet):
            continue
        name = alloc.memorylocations[0].name
        if alloc.kind == "ExternalInput":
            if name != part_name:
                in_names.append(name)
        elif alloc.kind == "ExternalOutput":
            out_names.append(name)
            shape = tuple(alloc.tensor_shape)
            dtype = mybir.dt.np(alloc.dtype)
            out_avals.append(jax.core.ShapedArray(shape, dtype))
            zero_outs.append(np.zeros(shape, dtype))
    n_params = len(in_names)
    all_names = in_names + out_names
    if part_name is not None:
        all_names = all_names + [part_name]
    donate = tuple(range(n_params, n_params + len(out_names)))

    def _body(*args):
        operands = list(args)
        if part_name is not None:
            operands.append(bass2jax.partition_id_tensor())
        outs = bass2jax._bass_exec_p.bind(
            *operands,
            out_avals=tuple(out_avals),
            in_names=tuple(all_names),
            out_names=tuple(out_names),
            lowering_input_output_aliases=(),
            sim_require_finite=True,
            sim_require_nnan=True,
            nc=nc,
        )
        return tuple(outs)

    devices = jax.devices()[:N_CORES]
    mesh = Mesh(np.asarray(devices), ("core",))
    specs = (PartitionSpec("core"),) * (n_params + len(out_names))
    sharded = jax.jit(
        shard_map(_body, mesh=mesh, in_specs=specs,
                  out_specs=(PartitionSpec("core"),) * len(out_names),
                  check_rep=False),
        donate_argnums=donate, keep_unused=True,
    )
    runner = dict(fn=sharded, in_names=in_names, out_names=out_names,
                  zero_outs=zero_outs, consts=consts, nc=nc)
    _CACHE["runner"] = runner
    return runner


def _concat_inputs(q, k, runner):
    """Per-core input dict -> globally concatenated arrays (axis 0)."""
    arrs = []
    for name in runner["in_names"]:
        if name == "q":
            arrs.append(q)
        elif name == "k":
            arrs.append(k)
        else:
            raise KeyError(name)
    return arrs


def kernel(q, k):
    q = np.ascontiguousarray(np.asarray(q, dtype=np.float32))
    k = np.ascontiguousarray(np.asarray(k, dtype=np.float32))
    assert q.shape == (BH_TOTAL, SEQ, D) and k.shape == (BH_TOTAL, SEQ, D)

    runner = _get_runner()
    # bh-shard across 8 cores: core c gets bh slice [4c, 4c+4). The global
    # concat layout [32, ...] already matches (shard_map splits axis 0).
    concat_in = _concat_inputs(q, k, runner)
    concat_zeros = [np.zeros((N_CORES * z.shape[0], *z.shape[1:]), z.dtype)
                    for z in runner["zero_outs"]]
    out_arrs = runner["fn"](*concat_in, *concat_zeros)
    out = np.asarray(out_arrs[0])          # [8*4, 128, 129]
    return np.ascontiguousarray(out.reshape(BH_TOTAL, NB, NJ))
